# revision 25
# baseline (speedup 1.0000x reference)
"""MoE layer (8 experts, top-2) on 8 TRN2 NeuronCores — expert parallelism.

Contract: kernel(**inputs) takes FULL inputs, returns FULL output.
Strategy:
  - Host computes the (tiny) gate: logits -> top-2 -> softmax, gathers tokens
    per expert (dispatch), and scatter-adds the scaled expert outputs back
    (combine).  Gate probs are applied on the host during the combine, so the
    device kernel is a plain per-expert FFN.
  - Capacity balancing: per-core columns C are split [0, a) + [a, C).  The A
    range runs the core's own expert; the B range runs spill-over tokens of an
    overloaded expert (second weight set), so C ~ max(1024, fragmentation)
    instead of the max expert load.  (C, a) come from a tiny search over the
    actual expert loads; single-set fallback when no packing helps.
  - Core kernel (bf16 operands, f32 psum): both stages stream the token dim
    as the matmul moving dimension, so PE cost is 512*C cycles exactly:
      phase 1: hT[f, t] = relu(w1.T @ xgt + b1)   (256*C PE cycles)
      phase 2: yT[d, t] = w2-tiles.T @ hT         (256*C PE cycles)
    hT ([128, 32, C] bf16) stays SBUF-resident; w1 streams per F-block in
    phase 1; w2 streams per D-tile in phase 2 from a host-preblocked layout
    ([dt, fi, fo*di]) so every DMA line is 8 KiB.
  - A PE "warmup" chain of dummy matmuls (on a memset tile, no DMA deps)
    runs the p-state clock ramp during the head DMA.

Shapes (hardcoded from the problem spec):
  x [2048, 2, 1024], gate_w [1024, 8], gate_b [8],
  w1 [8, 1024, 4096], b1 [8, 4096], w2 [8, 4096, 1024], b2 [8, 1024].
"""
import sys
import numpy as np

for _p in ("/opt/trn_rl_repo", "/root/.axon_site/_ro/trn_rl_repo"):
    if _p not in sys.path:
        sys.path.insert(0, _p)

import ml_dtypes
import concourse.bacc as bacc
import concourse.tile as tile
import concourse.mybir as mybir
from concourse import bass2jax, mybir as _mybir

N_EXPERTS = 8
TOP_K = 2
S, B, D, F = 2048, 2, 1024, 4096
P = 128
FB = 512                # F-block size streamed through SBUF in phase 1
NB = F // FB            # 8 F-blocks
FC = FB // P            # 4 F-partition-tiles per block
FO = F // P             # 32 F-partition-tiles total
DK = D // P             # 8 contraction tiles for phase 1
DT = D // P             # 8 output D-tiles for phase 2

_f32 = mybir.dt.float32
_bf16 = mybir.dt.bfloat16
_bf16_np = ml_dtypes.bfloat16

_NC_CACHE: dict = {}
_C_MAX = 1664           # max capacity per pass (SBUF budget bound)
LAST_DEVICE_NS = -1     # wall-clock of the last device dispatch (incl. transfers)
LAST_C = -1
LAST_A = -1
LAST_SLOTS = ()


def _c_chunks(C):
    """Split C into chunks <=512 (PSUM bank width in f32), remainder last."""
    out, pos = [], 0
    while C - pos >= 512:
        out.append((pos, 512))
        pos += 512
    if C - pos:
        out.append((pos, C - pos))
    return out


def _slot_assign(surpluses, sizes, n_cores):
    """DFS: per-expert slot counts (k_j slots of sizes[j]) covering each
    surplus, <= n_cores slots of each size total.  Returns per-expert count
    tuples or None."""
    import math
    order = [i for i in range(len(surpluses)) if surpluses[i] > 0]
    ns = len(sizes)

    def dfs(pos, used):
        if pos == len(order):
            return []
        s = surpluses[order[pos]]
        best = None
        # enumerate count tuples for this expert (small search space)
        def opts(rem, j):
            if j == ns - 1:
                k = 0 if rem <= 0 else math.ceil(rem / sizes[j])
                yield (k,)
                return
            kmax = n_cores - used[j]
            for k in range(0, kmax + 1):
                for rest in opts(rem - k * sizes[j], j + 1):
                    yield (k,) + rest
                if rem - k * sizes[j] <= 0:
                    break
        for ks in opts(s, 0):
            if all(used[j] + ks[j] <= n_cores for j in range(ns)):
                sub = dfs(pos + 1, tuple(used[j] + ks[j] for j in range(ns)))
                if sub is not None:
                    return [(order[pos], ks)] + sub
        return best

    return dfs(0, (0,) * ns)


def _pack(loads):
    """Pick (C, a, slots): per-core columns C = a + sum(slots); [0, a) runs
    the core's own expert, each spill slot one (possibly other) expert's
    overflow.  Searches 1- then 2-slot layouts for minimal C."""
    mx = max(loads)
    n = len(loads)
    lo = -(-sum(loads) // n)
    # 1-slot exhaustive
    best1 = None
    for C in range(-(-lo // 4) * 4, mx + 1, 4):
        for a in range(max(C - 512, 1), C):
            bsz = C - a
            if sum(-(-max(0, x - a) // bsz) for x in loads) <= n:
                best1 = (C, a, [bsz])
                break
        if best1:
            break
    # NOTE: a 2-slot (3 weight set) packing reaches C=1028 for these loads,
    # but the sim shows the 16MB of extra spill w1/w2 streaming costs more in
    # DMA tail pressure than the 3.4us the smaller C buys; 1-slot wins.
    best = best1
    if best is None or best[0] >= mx:
        C = -(-mx // 4) * 4
        return C, C, []
    return best


def _build(C, a, slots=(), passes=1, *, psum_bufs=8, w1_bufs=2, w2_bufs=2,
           y_bufs=2, n_warm=30, warm_w=128):
    """Trace + compile the per-core SPMD program.

    Columns [0, a) use weight set 0 (the core's own expert); each spill slot
    [a+o, a+o+s) uses its own weight set (inputs w1b/w1c, ...).
    passes>1 repeats the whole compute (same output) — used only for
    differential timing of the device kernel.
    """
    slots = tuple(slots)
    key = (C, a, slots, passes, psum_bufs, w1_bufs, w2_bufs, y_bufs,
           n_warm, warm_w)
    if key in _NC_CACHE:
        return _NC_CACHE[key]
    assert a + sum(slots) == C
    n_sets = 1 + len(slots)
    sfx = ["a", "b", "c"][:n_sets]
    nc = bacc.Bacc("TRN2", target_bir_lowering=False, debug=False,
                   enable_asserts=False, num_devices=8)
    xgt_d = nc.dram_tensor("xgt", (D, C), _bf16, kind="ExternalInput").ap()
    w1_d, b1_d, w2_d = [], [], []
    for s in sfx:
        w1_d.append(nc.dram_tensor(f"w1{s}", (D, F), _bf16,
                                   kind="ExternalInput").ap())
        b1_d.append(nc.dram_tensor(f"b1{s}", (P, FO), _f32,
                                   kind="ExternalInput").ap())
        # w2 host-preblocked: [dt*fi, fo*di] so per-(dt) DMA lines are 8 KiB
        w2_d.append(nc.dram_tensor(f"w2{s}", (DT * P, FO * P), _bf16,
                                   kind="ExternalInput").ap())
    yT_d = nc.dram_tensor("yT", (D, C), _f32, kind="ExternalOutput").ap()

    xgt_r = xgt_d.rearrange("(ko ki) c -> ki ko c", ki=P)      # [128, 8, C]
    w1_r = [t.rearrange("(ko ki) f -> ki ko f", ki=P) for t in w1_d]
    w2_r = [t.rearrange("(dt fi) fod -> fi dt fod", fi=P) for t in w2_d]
    yT_r = yT_d.rearrange("(do di) c -> di do c", di=P)        # [128, 8, C]

    # chunk = (col offset, size, set idx, offset within the set's hT tile)
    chunks = [(cs, csz, 0, cs) for (cs, csz) in _c_chunks(a)]
    off = a
    for gi, s in enumerate(slots):
        chunks.append((off, s, 1 + gi, 0))
        off += s
    spill_chunks = chunks[len(chunks) - len(slots):]

    with tile.TileContext(nc) as tc:
        with tc.tile_pool(name="const", bufs=1) as cpool, \
             tc.tile_pool(name="w1p", bufs=w1_bufs) as w1pool, \
             tc.tile_pool(name="w2p", bufs=w2_bufs) as w2pool, \
             tc.tile_pool(name="yp", bufs=y_bufs) as ypool, \
             tc.tile_pool(name="ps", bufs=psum_bufs, space="PSUM") as psum:
            xgt_sb = cpool.tile([P, DK, C], _bf16)
            b1_sb = [cpool.tile([P, FO], _f32, name=f"b1_{i}")
                     for i in range(n_sets)]
            # per-set hT tiles: phase-2 groups of one set must not pick up
            # whole-tile dependencies on another set's late relus
            hT_g = [cpool.tile([P, FO, a], _bf16, name="hT_0")]
            for gi, s in enumerate(slots):
                hT_g.append(cpool.tile([P, FO, s], _bf16, name=f"hT_{1+gi}"))

            if n_warm:
                # PE p-state warmup: memset a tile (no DMA dependency) and
                # chain dummy matmuls so the clock ramp runs concurrently
                # with the head DMA.  Result is never read.
                warm = cpool.tile([P, warm_w], _bf16)
                nc.vector.memset(warm[:], 0.0)
                wps = psum.tile([P, warm_w], _f32, tag="ps", name="warm_ps")
                for i in range(n_warm):
                    nc.tensor.matmul(wps[:], warm[:], warm[:],
                                     start=(i == 0), stop=(i == n_warm - 1))

            def relu(fb, fc, ch, ps):
                cs, csz, g, hcs = ch
                fcol = fb * FC + fc
                nc.scalar.activation(
                    hT_g[g][:, fcol, hcs:hcs + csz], ps[:, :csz],
                    mybir.ActivationFunctionType.Relu,
                    bias=b1_sb[g][:, fcol:fcol + 1], scale=1.0,
                )

            def stage1_groups(fb, w1_t, use_chunks, wave_head=False):
                groups = [(fc, ch) for ch in use_chunks for fc in range(FC)]
                if wave_head:
                    # dk-major waves: up to psum_bufs groups accumulate
                    # concurrently so the PE consumes each xgt[dk] as it lands
                    for ws in range(0, len(groups), psum_bufs):
                        wave = groups[ws:ws + psum_bufs]
                        pss = [psum.tile([P, 512], _f32, name=f"ps1h_{ws}_{i}",
                                         tag="ps")
                               for i in range(len(wave))]
                        for dk in range(DK):
                            for (fc, ch), ps in zip(wave, pss):
                                cs, csz, g, _ = ch
                                nc.tensor.matmul(
                                    ps[:, :csz],
                                    w1_t[g][:, dk, fc * P:(fc + 1) * P],
                                    xgt_sb[:, dk, cs:cs + csz],
                                    start=(dk == 0), stop=(dk == DK - 1),
                                )
                        for (fc, ch), ps in zip(wave, pss):
                            relu(fb, fc, ch, ps)
                else:
                    for (fc, ch) in groups:
                        cs, csz, g, _ = ch
                        ps = psum.tile([P, 512], _f32, tag="ps")
                        for dk in range(DK):
                            nc.tensor.matmul(
                                ps[:, :csz],
                                w1_t[g][:, dk, fc * P:(fc + 1) * P],
                                xgt_sb[:, dk, cs:cs + csz],
                                start=(dk == 0), stop=(dk == DK - 1),
                            )
                        relu(fb, fc, ch, ps)

            chunksA_g = chunks[:len(chunks) - len(slots)]

            def phase1(first_rep):
                """Spill work is deferred one section: w1b/w1c(fb) loads ride
                behind w1a(fb+1), and the spill groups run after A(fb+1) —
                keeps the large spill w1 loads out of the congested head DMA
                window and gives each one a full A-section to land."""
                def emit_spill_dmas(pfb):
                    tiles = {}
                    for g in range(1, n_sets):
                        t = w1pool.tile([P, DK, FB], _bf16, tag=f"w1_{g}",
                                        name="w1s_t")
                        nc.sync.dma_start(
                            t[:], w1_r[g][:, :, pfb * FB:(pfb + 1) * FB])
                        tiles[g] = t
                    return tiles

                for fb in range(NB):
                    w1a_t = w1pool.tile([P, DK, FB], _bf16, tag="w1a",
                                        name="w1a_t")
                    if first_rep and fb == 0:
                        # head: per-dk interleave of w1a-block0 and xgt so the
                        # PE starts on dk 0 while later dk slices stream in
                        for dk in range(DK):
                            nc.sync.dma_start(w1a_t[:, dk], w1_r[0][:, dk, 0:FB])
                            nc.sync.dma_start(xgt_sb[:, dk], xgt_r[:, dk])
                        for g in range(n_sets):
                            nc.sync.dma_start(b1_sb[g][:], b1_d[g])
                    else:
                        nc.sync.dma_start(
                            w1a_t[:], w1_r[0][:, :, fb * FB:(fb + 1) * FB])
                    if slots and fb >= 1:
                        sp_tiles = emit_spill_dmas(fb - 1)
                    stage1_groups(fb, [w1a_t], chunksA_g,
                                  wave_head=(first_rep and fb == 0))
                    if slots and fb >= 1:
                        sp = dict(sp_tiles)
                        sp[0] = None
                        stage1_groups(fb - 1, sp, spill_chunks)
                if slots:
                    sp = dict(emit_spill_dmas(NB - 1))
                    sp[0] = None
                    stage1_groups(NB - 1, sp, spill_chunks)

            def phase2():
                """yT[dt, :] = sum_fo w2[fo, dt].T @ hT[fo, :], streamed out."""
                for dt in range(DT):
                    w2_t = []
                    for g in range(n_sets):
                        t = w2pool.tile([P, FO * P], _bf16, tag=f"w2_{g}",
                                        name="w2_t")
                        nc.sync.dma_start(t[:], w2_r[g][:, dt, :])
                        w2_t.append(t)
                    for (cs, csz, g, hcs) in chunks:
                        ps2 = psum.tile([P, 512], _f32, tag="ps")
                        for fo in range(FO):
                            nc.tensor.matmul(
                                ps2[:, :csz],
                                w2_t[g][:, fo * P:(fo + 1) * P],
                                hT_g[g][:, fo, hcs:hcs + csz],
                                start=(fo == 0), stop=(fo == FO - 1),
                            )
                        yt = ypool.tile([P, 512], _f32)
                        nc.vector.tensor_copy(yt[:, :csz], ps2[:, :csz])
                        nc.sync.dma_start(yT_r[:, dt, cs:cs + csz], yt[:, :csz])

            for rep in range(passes):
                phase1(rep == 0)
                phase2()
    nc.compile()
    _NC_CACHE[key] = nc
    return nc


class _Runner:
    """Persistent jitted SPMD executor for a compiled Bacc program.

    Mirrors bass2jax.run_bass_via_pjrt but keeps the jitted callable so
    repeat calls skip retracing/recompiling.
    """

    def __init__(self, nc, n_cores):
        import jax
        from jax.sharding import Mesh, PartitionSpec
        from jax.experimental.shard_map import shard_map

        bass2jax.install_neuronx_cc_hook()
        self.nc = nc
        self.n_cores = n_cores
        in_names, out_names, out_avals = [], [], []
        for alloc in nc.m.functions[0].allocations:
            if not isinstance(alloc, _mybir.MemoryLocationSet):
                continue
            name = alloc.memorylocations[0].name
            if alloc.kind == "ExternalInput":
                in_names.append(name)
            elif alloc.kind == "ExternalOutput":
                out_names.append(name)
                out_avals.append(jax.core.ShapedArray(
                    tuple(alloc.tensor_shape), _mybir.dt.np(alloc.dtype)))
        partition_name = nc.partition_id_tensor.name if nc.partition_id_tensor else None
        in_names = [n for n in in_names if n != partition_name]
        all_names = in_names + out_names + ([partition_name] if partition_name else [])
        self.in_names, self.out_names, self.out_avals = in_names, out_names, out_avals
        self._all_names, self._partition_name = all_names, partition_name
        n_params = len(in_names)

        def _body(*args):
            operands = list(args)
            if partition_name is not None:
                operands.append(bass2jax.partition_id_tensor())
            outs = bass2jax._bass_exec_p.bind(
                *operands,
                out_avals=tuple(out_avals),
                in_names=tuple(all_names),
                out_names=tuple(out_names),
                lowering_input_output_aliases=(),
                sim_require_finite=False,
                sim_require_nnan=False,
                nc=nc,
            )
            return tuple(outs)

        devices = jax.devices()[:n_cores]
        mesh = Mesh(np.asarray(devices), ("core",))
        n_outs = len(out_names)
        self._fn = jax.jit(
            shard_map(_body, mesh=mesh,
                      in_specs=(PartitionSpec("core"),) * (n_params + n_outs),
                      out_specs=(PartitionSpec("core"),) * n_outs,
                      check_rep=False),
            donate_argnums=tuple(range(n_params, n_params + n_outs)),
            keep_unused=True,
        )
        self._jax = jax

    def concat_inputs(self, in_maps):
        return [np.concatenate([np.asarray(m[name]) for m in in_maps], axis=0)
                for name in self.in_names]

    def zero_outs(self):
        jnp = self._jax.numpy
        return [jnp.zeros((self.n_cores * a.shape[0], *a.shape[1:]), a.dtype)
                for a in self.out_avals]

    def run_raw(self, concat_in, zouts):
        outs = self._fn(*concat_in, *zouts)
        self._jax.block_until_ready(outs)
        return outs

    def run(self, in_maps):
        outs = self.run_raw(self.concat_inputs(in_maps), self.zero_outs())
        return [
            {name: np.asarray(outs[i]).reshape(self.n_cores, *self.out_avals[i].shape)[c]
             for i, name in enumerate(self.out_names)}
            for c in range(self.n_cores)
        ]


_RUNNER_CACHE: dict = {}


def _runner(C, a, slots=(), passes=1):
    key = (C, a, tuple(slots), passes)
    if key not in _RUNNER_CACHE:
        _RUNNER_CACHE[key] = _Runner(_build(C, a, slots, passes), N_EXPERTS)
    return _RUNNER_CACHE[key]


def _route(x2d, gate_w, gate_b):
    """Host gate: returns per-token top-2 expert ids and softmax probs (fp32)."""
    logits = x2d.astype(np.float64) @ gate_w.astype(np.float64) + gate_b.astype(np.float64)
    order = np.argsort(-logits, axis=-1, kind="stable")
    top2 = order[:, :TOP_K]                               # [T, 2]
    l = np.take_along_axis(logits, top2, axis=-1)         # [T, 2]
    m = l.max(axis=-1, keepdims=True)
    e = np.exp(l - m)
    p = (e / e.sum(axis=-1, keepdims=True)).astype(np.float32)
    return top2, p


def _block_w2(w2_e_bf):
    """[F, D] -> [dt*fi, fo*di] so per-dt DMA lines are contiguous 8 KiB."""
    return np.ascontiguousarray(
        w2_e_bf.reshape(FO, P, DT, P).transpose(2, 1, 0, 3).reshape(DT * P, FO * P))


def _b1t(b1_e):
    return np.ascontiguousarray(b1_e.astype(np.float32).reshape(FO, P).T)


def prepare(x, gate_w, gate_b, w1, b1, w2):
    """Routing + packing + per-core input maps.  Returns (in_maps, metas,
    C, a, slots) where metas[c] = [(col_off, ix, probs), ...] scatter jobs."""
    T = S * B
    x2d = np.ascontiguousarray(np.asarray(x, np.float32).reshape(T, D))
    top2, p = _route(x2d, np.asarray(gate_w, np.float32),
                     np.asarray(gate_b, np.float32))
    idx_lists = []
    for e in range(N_EXPERTS):
        sel = np.nonzero(top2 == e)          # (token_idx, slot_idx)
        idx_lists.append((sel[0], p[sel[0], sel[1]]))
    loads = [len(ix) for ix, _ in idx_lists]
    C, a, slots = _pack(loads)

    # spill assignment: per slot kind j, per core at most one piece
    # (expert, lo, hi); greedy from the _slot_assign counts
    n_spill = len(slots)
    spill = [[None] * n_spill for _ in range(N_EXPERTS)]
    if n_spill:
        sur = [max(0, n - a) for n in loads]
        counts = _slot_assign(sur, list(slots), N_EXPERTS)
        assert counts is not None
        free = [list(range(N_EXPERTS)) for _ in range(n_spill)]
        for e, ks in counts:
            lo = a
            for j in range(n_spill):
                for _ in range(ks[j]):
                    hi = min(lo + slots[j], loads[e])
                    if lo >= hi:
                        continue
                    c = e if e in free[j] else free[j][0]
                    free[j].remove(c)
                    spill[c][j] = (e, lo, hi)
                    lo = hi
            assert lo >= loads[e]

    xT_bf = np.ascontiguousarray(x2d.T.astype(_bf16_np))  # [D, T] bf16
    w1_bf = [np.ascontiguousarray(np.asarray(w1[e]).astype(_bf16_np))
             for e in range(N_EXPERTS)]
    w2_blk = [_block_w2(np.asarray(w2[e]).astype(_bf16_np))
              for e in range(N_EXPERTS)]
    b1_t = [_b1t(np.asarray(b1[e])) for e in range(N_EXPERTS)]

    sfx = ["a", "b", "c"]
    in_maps, metas = [], []
    for c in range(N_EXPERTS):
        ix_a, p_a = idx_lists[c]
        ix_a, p_a = ix_a[:a], p_a[:a]
        xgt = np.zeros((D, C), dtype=_bf16_np)
        xgt[:, :len(ix_a)] = xT_bf[:, ix_a]
        m = {"xgt": xgt, "w1a": w1_bf[c], "b1a": b1_t[c], "w2a": w2_blk[c]}
        jobs = [(0, ix_a, p_a)] if len(ix_a) else []
        off = a
        for j in range(n_spill):
            e = spill[c][j][0] if spill[c][j] else c
            if spill[c][j]:
                _, lo, hi = spill[c][j]
                ix_s, p_s = idx_lists[e][0][lo:hi], idx_lists[e][1][lo:hi]
                xgt[:, off:off + hi - lo] = xT_bf[:, ix_s]
                jobs.append((off, ix_s, p_s))
            m.update({f"w1{sfx[1+j]}": w1_bf[e], f"b1{sfx[1+j]}": b1_t[e],
                      f"w2{sfx[1+j]}": w2_blk[e]})
            off += slots[j]
        in_maps.append(m)
        metas.append(jobs)
    return in_maps, metas, C, a, slots


def kernel(x, gate_w, gate_b, w1, b1, w2, b2):
    in_maps, metas, C, a, slots = prepare(x, gate_w, gate_b, w1, b1, w2)
    global LAST_C, LAST_A, LAST_SLOTS
    LAST_C, LAST_A, LAST_SLOTS = C, a, slots
    runner = _runner(C, a, slots)

    import time as _time
    _t0 = _time.time()
    results = runner.run(in_maps)
    global LAST_DEVICE_NS
    LAST_DEVICE_NS = int((_time.time() - _t0) * 1e9)

    T = S * B
    out2d = np.zeros((T, D), dtype=np.float32)
    for c in range(N_EXPERTS):
        yT = results[c]["yT"]
        for (off, ix, pr) in metas[c]:
            # combine: scale by gate prob during the scatter-add
            out2d[ix] += pr[:, None] * yT[:, off:off + len(ix)].T

    b2 = np.asarray(b2, np.float32)
    if np.any(b2):
        x2d = np.asarray(x, np.float32).reshape(T, D)
        top2, p = _route(x2d, np.asarray(gate_w, np.float32),
                         np.asarray(gate_b, np.float32))
        comb = np.zeros((T, N_EXPERTS), dtype=np.float32)
        np.put_along_axis(comb, top2, p, axis=-1)
        out2d += comb @ b2
    return out2d.reshape(S, B, D)


# revision 31
# speedup vs baseline: 1.0128x; 1.0128x over previous
"""MoE layer (8 experts, top-2) on 8 TRN2 NeuronCores — expert parallelism.

Contract: kernel(**inputs) takes FULL inputs, returns FULL output.
Strategy:
  - Host computes the (tiny) gate: logits -> top-2 -> softmax, gathers tokens
    per expert (dispatch), and scatter-adds the scaled expert outputs back
    (combine).  Gate probs are applied on the host during the combine, so the
    device kernel is a plain per-expert FFN.
  - Capacity balancing: per-core columns C are split [0, a) + [a, C).  The A
    range runs the core's own expert; the B range runs spill-over tokens of an
    overloaded expert (second weight set), so C ~ max(1024, fragmentation)
    instead of the max expert load.  (C, a) come from a tiny search over the
    actual expert loads; single-set fallback when no packing helps.
  - Core kernel (bf16 operands, f32 psum): both stages stream the token dim
    as the matmul moving dimension, so PE cost is 512*C cycles exactly:
      phase 1: hT[f, t] = relu(w1.T @ xgt + b1)   (256*C PE cycles)
      phase 2: yT[d, t] = w2-tiles.T @ hT         (256*C PE cycles)
    hT ([128, 32, C] bf16) stays SBUF-resident; w1 streams per F-block in
    phase 1; w2 streams per D-tile in phase 2 from a host-preblocked layout
    ([dt, fi, fo*di]) so every DMA line is 8 KiB.
  - A PE "warmup" chain of dummy matmuls (on a memset tile, no DMA deps)
    runs the p-state clock ramp during the head DMA.

Shapes (hardcoded from the problem spec):
  x [2048, 2, 1024], gate_w [1024, 8], gate_b [8],
  w1 [8, 1024, 4096], b1 [8, 4096], w2 [8, 4096, 1024], b2 [8, 1024].
"""
import sys
import numpy as np

for _p in ("/opt/trn_rl_repo", "/root/.axon_site/_ro/trn_rl_repo"):
    if _p not in sys.path:
        sys.path.insert(0, _p)

import ml_dtypes
import concourse.bacc as bacc
import concourse.tile as tile
import concourse.mybir as mybir
from concourse import bass2jax, mybir as _mybir

N_EXPERTS = 8
TOP_K = 2
S, B, D, F = 2048, 2, 1024, 4096
P = 128
FB = 512                # F-block size streamed through SBUF in phase 1
NB = F // FB            # 8 F-blocks
FC = FB // P            # 4 F-partition-tiles per block
FO = F // P             # 32 F-partition-tiles total
DK = D // P             # 8 contraction tiles for phase 1
DT = D // P             # 8 output D-tiles for phase 2

_f32 = mybir.dt.float32
_bf16 = mybir.dt.bfloat16
_bf16_np = ml_dtypes.bfloat16

_NC_CACHE: dict = {}
_C_MAX = 1664           # max capacity per pass (SBUF budget bound)
LAST_DEVICE_NS = -1     # wall-clock of the last device dispatch (incl. transfers)
LAST_C = -1
LAST_A = -1
LAST_SLOTS = ()


def _c_chunks(C):
    """Split C into chunks <=512 (PSUM bank width in f32), remainder last."""
    out, pos = [], 0
    while C - pos >= 512:
        out.append((pos, 512))
        pos += 512
    if C - pos:
        out.append((pos, C - pos))
    return out


def _slot_assign(surpluses, sizes, n_cores):
    """DFS: per-expert slot counts (k_j slots of sizes[j]) covering each
    surplus, <= n_cores slots of each size total.  Returns per-expert count
    tuples or None."""
    import math
    order = [i for i in range(len(surpluses)) if surpluses[i] > 0]
    ns = len(sizes)

    def dfs(pos, used):
        if pos == len(order):
            return []
        s = surpluses[order[pos]]
        best = None
        # enumerate count tuples for this expert (small search space)
        def opts(rem, j):
            if j == ns - 1:
                k = 0 if rem <= 0 else math.ceil(rem / sizes[j])
                yield (k,)
                return
            kmax = n_cores - used[j]
            for k in range(0, kmax + 1):
                for rest in opts(rem - k * sizes[j], j + 1):
                    yield (k,) + rest
                if rem - k * sizes[j] <= 0:
                    break
        for ks in opts(s, 0):
            if all(used[j] + ks[j] <= n_cores for j in range(ns)):
                sub = dfs(pos + 1, tuple(used[j] + ks[j] for j in range(ns)))
                if sub is not None:
                    return [(order[pos], ks)] + sub
        return best

    return dfs(0, (0,) * ns)


def _pack(loads):
    """Pick (C, a, slots): per-core columns C = a + sum(slots); [0, a) runs
    the core's own expert, each spill slot one (possibly other) expert's
    overflow.  Searches 1- then 2-slot layouts for minimal C."""
    mx = max(loads)
    n = len(loads)
    lo = -(-sum(loads) // n)
    # 1-slot exhaustive
    best1 = None
    for C in range(-(-lo // 4) * 4, mx + 1, 4):
        for a in range(max(C - 512, 1), C):
            bsz = C - a
            if sum(-(-max(0, x - a) // bsz) for x in loads) <= n:
                best1 = (C, a, [bsz])
                break
        if best1:
            break
    # 2-slot search below best1 (spill-w2 prefetch in _build keeps the extra
    # spill streams off the phase-2 tail)
    best = best1
    cap = best1[0] if best1 else mx
    for C in range(-(-lo // 4) * 4, cap, 4):
        found = None
        for a in range(C - 2, max(C - 1024, 0), -1):
            rest = C - a
            for s1 in range(1, rest):
                s2 = rest - s1
                if s1 > s2:
                    continue
                sur = [max(0, x - a) for x in loads]
                if _slot_assign(sur, [s2, s1], n) is not None:
                    found = (C, a, [s2, s1])
                    break
            if found:
                break
        if found:
            best = found
            break
    if best is None or best[0] >= mx:
        C = -(-mx // 4) * 4
        return C, C, []
    return best


def _build(C, a, slots=(), passes=1, *, psum_bufs=8, w1_bufs=2, w2_bufs=2,
           y_bufs=6, n_warm=30, warm_w=128):
    """Trace + compile the per-core SPMD program.

    Columns [0, a) use weight set 0 (the core's own expert); each spill slot
    [a+o, a+o+s) uses its own weight set (inputs w1b/w1c, ...).
    passes>1 repeats the whole compute (same output) — used only for
    differential timing of the device kernel.
    """
    slots = tuple(slots)
    key = (C, a, slots, passes, psum_bufs, w1_bufs, w2_bufs, y_bufs,
           n_warm, warm_w)
    if key in _NC_CACHE:
        return _NC_CACHE[key]
    assert a + sum(slots) == C
    n_sets = 1 + len(slots)
    sfx = ["a", "b", "c"][:n_sets]
    nc = bacc.Bacc("TRN2", target_bir_lowering=False, debug=False,
                   enable_asserts=False, num_devices=8)
    xgt_d = nc.dram_tensor("xgt", (D, C), _bf16, kind="ExternalInput").ap()
    w1_d, w2_d = [], []
    # all sets' b1 packed into one tensor -> one head DMA (HWDGE is 625ns
    # per DMA and the early queue window is congested)
    b1_d = nc.dram_tensor("b1p", (P, n_sets * FO), _f32,
                          kind="ExternalInput").ap()
    for s in sfx:
        w1_d.append(nc.dram_tensor(f"w1{s}", (D, F), _bf16,
                                   kind="ExternalInput").ap())
        # w2 host-preblocked: [dt*fi, fo*di] so per-(dt) DMA lines are 8 KiB
        w2_d.append(nc.dram_tensor(f"w2{s}", (DT * P, FO * P), _bf16,
                                   kind="ExternalInput").ap())
    yT_d = nc.dram_tensor("yT", (D, C), _f32, kind="ExternalOutput").ap()

    xgt_r = xgt_d.rearrange("(ko ki) c -> ki ko c", ki=P)      # [128, 8, C]
    w1_r = [t.rearrange("(ko ki) f -> ki ko f", ki=P) for t in w1_d]
    w2_r = [t.rearrange("(dt fi) fod -> fi dt fod", fi=P) for t in w2_d]
    yT_r = yT_d.rearrange("(do di) c -> di do c", di=P)        # [128, 8, C]

    # chunk = (col offset, size, set idx, offset within the set's hT tile)
    chunks = [(cs, csz, 0, cs) for (cs, csz) in _c_chunks(a)]
    off = a
    for gi, s in enumerate(slots):
        chunks.append((off, s, 1 + gi, 0))
        off += s
    spill_chunks = chunks[len(chunks) - len(slots):]

    with tile.TileContext(nc) as tc:
        with tc.tile_pool(name="const", bufs=1) as cpool, \
             tc.tile_pool(name="w1p", bufs=w1_bufs) as w1pool, \
             tc.tile_pool(name="w2p", bufs=w2_bufs) as w2pool, \
             tc.tile_pool(name="w2s", bufs=4) as w2spool, \
             tc.tile_pool(name="yp", bufs=y_bufs) as ypool, \
             tc.tile_pool(name="ps", bufs=psum_bufs, space="PSUM") as psum:
            xgt_sb = cpool.tile([P, DK, C], _bf16)
            b1_sb = cpool.tile([P, n_sets * FO], _f32)
            # per-set hT tiles: phase-2 groups of one set must not pick up
            # whole-tile dependencies on another set's late relus
            hT_g = [cpool.tile([P, FO, a], _bf16, name="hT_0")]
            for gi, s in enumerate(slots):
                hT_g.append(cpool.tile([P, FO, s], _bf16, name=f"hT_{1+gi}"))

            if n_warm:
                # PE p-state warmup: memset a tile (no DMA dependency) and
                # chain dummy matmuls so the clock ramp runs concurrently
                # with the head DMA.  Result is never read.
                warm = cpool.tile([P, warm_w], _bf16)
                nc.vector.memset(warm[:], 0.0)
                wps = psum.tile([P, warm_w], _f32, tag="ps", name="warm_ps")
                for i in range(n_warm):
                    nc.tensor.matmul(wps[:], warm[:], warm[:],
                                     start=(i == 0), stop=(i == n_warm - 1))

            def relu(fb, fc, ch, ps):
                cs, csz, g, hcs = ch
                fcol = fb * FC + fc
                nc.scalar.activation(
                    hT_g[g][:, fcol, hcs:hcs + csz], ps[:, :csz],
                    mybir.ActivationFunctionType.Relu,
                    bias=b1_sb[:, g * FO + fcol:g * FO + fcol + 1], scale=1.0,
                )

            def stage1_groups(fb, w1_t, use_chunks, wave_head=False):
                groups = [(fc, ch) for ch in use_chunks for fc in range(FC)]
                if wave_head:
                    # dk-major waves: up to psum_bufs groups accumulate
                    # concurrently so the PE consumes each xgt[dk] as it lands
                    for ws in range(0, len(groups), psum_bufs):
                        wave = groups[ws:ws + psum_bufs]
                        pss = [psum.tile([P, 512], _f32, name=f"ps1h_{ws}_{i}",
                                         tag="ps")
                               for i in range(len(wave))]
                        for dk in range(DK):
                            for (fc, ch), ps in zip(wave, pss):
                                cs, csz, g, _ = ch
                                nc.tensor.matmul(
                                    ps[:, :csz],
                                    w1_t[g][:, dk, fc * P:(fc + 1) * P],
                                    xgt_sb[:, dk, cs:cs + csz],
                                    start=(dk == 0), stop=(dk == DK - 1),
                                )
                        for (fc, ch), ps in zip(wave, pss):
                            relu(fb, fc, ch, ps)
                else:
                    for (fc, ch) in groups:
                        cs, csz, g, _ = ch
                        ps = psum.tile([P, 512], _f32, tag="ps")
                        for dk in range(DK):
                            nc.tensor.matmul(
                                ps[:, :csz],
                                w1_t[g][:, dk, fc * P:(fc + 1) * P],
                                xgt_sb[:, dk, cs:cs + csz],
                                start=(dk == 0), stop=(dk == DK - 1),
                            )
                        relu(fb, fc, ch, ps)

            chunksA_g = chunks[:len(chunks) - len(slots)]

            # spill-set w2 streams through a FIFO pool at ~2-dt prefetch
            # distance; the first PF_DT dts' tiles load during late phase-1
            # sections (the DMA queue has slack there), keeping the spill
            # streams off the phase-2 tail
            PF_DT = 2 if slots else 0
            w2s_tiles = {}

            def w2s_fetch(g, dt):
                t = w2spool.tile([P, FO * P], _bf16, tag="w2s", name="w2s_t")
                nc.sync.dma_start(t[:], w2_r[g][:, dt, :])
                w2s_tiles[(g, dt)] = t

            def phase1(first_rep):
                """Spill work is deferred one section: w1b/w1c(fb) loads ride
                behind w1a(fb+1), and the spill groups run after A(fb+1) —
                keeps the large spill w1 loads out of the congested head DMA
                window and gives each one a full A-section to land."""
                def emit_spill_dmas(pfb):
                    tiles = {}
                    for g in range(1, n_sets):
                        t = w1pool.tile([P, DK, FB], _bf16, tag=f"w1_{g}",
                                        name="w1s_t")
                        nc.sync.dma_start(
                            t[:], w1_r[g][:, :, pfb * FB:(pfb + 1) * FB])
                        tiles[g] = t
                    return tiles

                for fb in range(NB):
                    w1a_t = w1pool.tile([P, DK, FB], _bf16, tag="w1a",
                                        name="w1a_t")
                    if first_rep and fb == 0:
                        # head: per-dk interleave of w1a-block0 and xgt so the
                        # PE starts on dk 0 while later dk slices stream in
                        for dk in range(DK):
                            nc.sync.dma_start(w1a_t[:, dk], w1_r[0][:, dk, 0:FB])
                            nc.sync.dma_start(xgt_sb[:, dk], xgt_r[:, dk])
                        nc.sync.dma_start(b1_sb[:], b1_d)
                    else:
                        nc.sync.dma_start(
                            w1a_t[:], w1_r[0][:, :, fb * FB:(fb + 1) * FB])
                    if slots and fb >= 1:
                        sp_tiles = emit_spill_dmas(fb - 1)
                    pf = fb - (NB - PF_DT * len(slots))
                    if 0 <= pf < PF_DT * len(slots):
                        w2s_fetch(1 + pf % len(slots), pf // len(slots))
                    stage1_groups(fb, [w1a_t], chunksA_g,
                                  wave_head=(first_rep and fb == 0))
                    if slots and fb >= 1:
                        sp = dict(sp_tiles)
                        sp[0] = None
                        stage1_groups(fb - 1, sp, spill_chunks)
                if slots:
                    sp = dict(emit_spill_dmas(NB - 1))
                    sp[0] = None
                    stage1_groups(NB - 1, sp, spill_chunks)

            def phase2():
                """yT[dt, :] = sum_fo w2[fo, dt].T @ hT[fo, :], streamed out."""
                for dt in range(DT):
                    w2a_t = w2pool.tile([P, FO * P], _bf16, tag="w2_0",
                                        name="w2a_t2")
                    nc.sync.dma_start(w2a_t[:], w2_r[0][:, dt, :])
                    if dt + PF_DT < DT:
                        for g in range(1, n_sets):
                            w2s_fetch(g, dt + PF_DT)
                    w2_t = [w2a_t] + [w2s_tiles.pop((g, dt))
                                      for g in range(1, n_sets)]
                    for (cs, csz, g, hcs) in chunks:
                        ps2 = psum.tile([P, 512], _f32, tag="ps")
                        for fo in range(FO):
                            nc.tensor.matmul(
                                ps2[:, :csz],
                                w2_t[g][:, fo * P:(fo + 1) * P],
                                hT_g[g][:, fo, hcs:hcs + csz],
                                start=(fo == 0), stop=(fo == FO - 1),
                            )
                        yt = ypool.tile([P, 512], _f32)
                        nc.vector.tensor_copy(yt[:, :csz], ps2[:, :csz])
                        nc.sync.dma_start(yT_r[:, dt, cs:cs + csz], yt[:, :csz])

            for rep in range(passes):
                phase1(rep == 0)
                phase2()
    nc.compile()
    _NC_CACHE[key] = nc
    return nc


class _Runner:
    """Persistent jitted SPMD executor for a compiled Bacc program.

    Mirrors bass2jax.run_bass_via_pjrt but keeps the jitted callable so
    repeat calls skip retracing/recompiling.
    """

    def __init__(self, nc, n_cores):
        import jax
        from jax.sharding import Mesh, PartitionSpec
        from jax.experimental.shard_map import shard_map

        bass2jax.install_neuronx_cc_hook()
        self.nc = nc
        self.n_cores = n_cores
        in_names, out_names, out_avals = [], [], []
        for alloc in nc.m.functions[0].allocations:
            if not isinstance(alloc, _mybir.MemoryLocationSet):
                continue
            name = alloc.memorylocations[0].name
            if alloc.kind == "ExternalInput":
                in_names.append(name)
            elif alloc.kind == "ExternalOutput":
                out_names.append(name)
                out_avals.append(jax.core.ShapedArray(
                    tuple(alloc.tensor_shape), _mybir.dt.np(alloc.dtype)))
        partition_name = nc.partition_id_tensor.name if nc.partition_id_tensor else None
        in_names = [n for n in in_names if n != partition_name]
        all_names = in_names + out_names + ([partition_name] if partition_name else [])
        self.in_names, self.out_names, self.out_avals = in_names, out_names, out_avals
        self._all_names, self._partition_name = all_names, partition_name
        n_params = len(in_names)

        def _body(*args):
            operands = list(args)
            if partition_name is not None:
                operands.append(bass2jax.partition_id_tensor())
            outs = bass2jax._bass_exec_p.bind(
                *operands,
                out_avals=tuple(out_avals),
                in_names=tuple(all_names),
                out_names=tuple(out_names),
                lowering_input_output_aliases=(),
                sim_require_finite=False,
                sim_require_nnan=False,
                nc=nc,
            )
            return tuple(outs)

        devices = jax.devices()[:n_cores]
        mesh = Mesh(np.asarray(devices), ("core",))
        n_outs = len(out_names)
        self._fn = jax.jit(
            shard_map(_body, mesh=mesh,
                      in_specs=(PartitionSpec("core"),) * (n_params + n_outs),
                      out_specs=(PartitionSpec("core"),) * n_outs,
                      check_rep=False),
            donate_argnums=tuple(range(n_params, n_params + n_outs)),
            keep_unused=True,
        )
        self._jax = jax

    def concat_inputs(self, in_maps):
        return [np.concatenate([np.asarray(m[name]) for m in in_maps], axis=0)
                for name in self.in_names]

    def zero_outs(self):
        jnp = self._jax.numpy
        return [jnp.zeros((self.n_cores * a.shape[0], *a.shape[1:]), a.dtype)
                for a in self.out_avals]

    def run_raw(self, concat_in, zouts):
        outs = self._fn(*concat_in, *zouts)
        self._jax.block_until_ready(outs)
        return outs

    def run(self, in_maps):
        outs = self.run_raw(self.concat_inputs(in_maps), self.zero_outs())
        return [
            {name: np.asarray(outs[i]).reshape(self.n_cores, *self.out_avals[i].shape)[c]
             for i, name in enumerate(self.out_names)}
            for c in range(self.n_cores)
        ]


_RUNNER_CACHE: dict = {}


def _runner(C, a, slots=(), passes=1):
    key = (C, a, tuple(slots), passes)
    if key not in _RUNNER_CACHE:
        _RUNNER_CACHE[key] = _Runner(_build(C, a, slots, passes), N_EXPERTS)
    return _RUNNER_CACHE[key]


def _route(x2d, gate_w, gate_b):
    """Host gate: returns per-token top-2 expert ids and softmax probs (fp32)."""
    logits = x2d.astype(np.float64) @ gate_w.astype(np.float64) + gate_b.astype(np.float64)
    order = np.argsort(-logits, axis=-1, kind="stable")
    top2 = order[:, :TOP_K]                               # [T, 2]
    l = np.take_along_axis(logits, top2, axis=-1)         # [T, 2]
    m = l.max(axis=-1, keepdims=True)
    e = np.exp(l - m)
    p = (e / e.sum(axis=-1, keepdims=True)).astype(np.float32)
    return top2, p


def _block_w2(w2_e_bf):
    """[F, D] -> [dt*fi, fo*di] so per-dt DMA lines are contiguous 8 KiB."""
    return np.ascontiguousarray(
        w2_e_bf.reshape(FO, P, DT, P).transpose(2, 1, 0, 3).reshape(DT * P, FO * P))


def _b1t(b1_e):
    return np.ascontiguousarray(b1_e.astype(np.float32).reshape(FO, P).T)


def prepare(x, gate_w, gate_b, w1, b1, w2):
    """Routing + packing + per-core input maps.  Returns (in_maps, metas,
    C, a, slots) where metas[c] = [(col_off, ix, probs), ...] scatter jobs."""
    T = S * B
    x2d = np.ascontiguousarray(np.asarray(x, np.float32).reshape(T, D))
    top2, p = _route(x2d, np.asarray(gate_w, np.float32),
                     np.asarray(gate_b, np.float32))
    idx_lists = []
    for e in range(N_EXPERTS):
        sel = np.nonzero(top2 == e)          # (token_idx, slot_idx)
        idx_lists.append((sel[0], p[sel[0], sel[1]]))
    loads = [len(ix) for ix, _ in idx_lists]
    C, a, slots = _pack(loads)

    # spill assignment: per slot kind j, per core at most one piece
    # (expert, lo, hi); greedy from the _slot_assign counts
    n_spill = len(slots)
    spill = [[None] * n_spill for _ in range(N_EXPERTS)]
    if n_spill:
        sur = [max(0, n - a) for n in loads]
        counts = _slot_assign(sur, list(slots), N_EXPERTS)
        assert counts is not None
        free = [list(range(N_EXPERTS)) for _ in range(n_spill)]
        for e, ks in counts:
            lo = a
            for j in range(n_spill):
                for _ in range(ks[j]):
                    hi = min(lo + slots[j], loads[e])
                    if lo >= hi:
                        continue
                    c = e if e in free[j] else free[j][0]
                    free[j].remove(c)
                    spill[c][j] = (e, lo, hi)
                    lo = hi
            assert lo >= loads[e]

    xT_bf = np.ascontiguousarray(x2d.T.astype(_bf16_np))  # [D, T] bf16
    w1_bf = [np.ascontiguousarray(np.asarray(w1[e]).astype(_bf16_np))
             for e in range(N_EXPERTS)]
    w2_blk = [_block_w2(np.asarray(w2[e]).astype(_bf16_np))
              for e in range(N_EXPERTS)]
    b1_t = [_b1t(np.asarray(b1[e])) for e in range(N_EXPERTS)]

    sfx = ["a", "b", "c"]
    in_maps, metas = [], []
    for c in range(N_EXPERTS):
        ix_a, p_a = idx_lists[c]
        ix_a, p_a = ix_a[:a], p_a[:a]
        xgt = np.zeros((D, C), dtype=_bf16_np)
        xgt[:, :len(ix_a)] = xT_bf[:, ix_a]
        m = {"xgt": xgt, "w1a": w1_bf[c], "w2a": w2_blk[c]}
        b1_parts = [b1_t[c]]
        jobs = [(0, ix_a, p_a)] if len(ix_a) else []
        off = a
        for j in range(n_spill):
            e = spill[c][j][0] if spill[c][j] else c
            if spill[c][j]:
                _, lo, hi = spill[c][j]
                ix_s, p_s = idx_lists[e][0][lo:hi], idx_lists[e][1][lo:hi]
                xgt[:, off:off + hi - lo] = xT_bf[:, ix_s]
                jobs.append((off, ix_s, p_s))
            m.update({f"w1{sfx[1+j]}": w1_bf[e], f"w2{sfx[1+j]}": w2_blk[e]})
            b1_parts.append(b1_t[e])
            off += slots[j]
        m["b1p"] = np.ascontiguousarray(np.concatenate(b1_parts, axis=1))
        in_maps.append(m)
        metas.append(jobs)
    return in_maps, metas, C, a, slots


def kernel(x, gate_w, gate_b, w1, b1, w2, b2):
    in_maps, metas, C, a, slots = prepare(x, gate_w, gate_b, w1, b1, w2)
    global LAST_C, LAST_A, LAST_SLOTS
    LAST_C, LAST_A, LAST_SLOTS = C, a, slots
    runner = _runner(C, a, slots)

    import time as _time
    _t0 = _time.time()
    results = runner.run(in_maps)
    global LAST_DEVICE_NS
    LAST_DEVICE_NS = int((_time.time() - _t0) * 1e9)

    T = S * B
    out2d = np.zeros((T, D), dtype=np.float32)
    for c in range(N_EXPERTS):
        yT = results[c]["yT"]
        for (off, ix, pr) in metas[c]:
            # combine: scale by gate prob during the scatter-add
            out2d[ix] += pr[:, None] * yT[:, off:off + len(ix)].T

    b2 = np.asarray(b2, np.float32)
    if np.any(b2):
        x2d = np.asarray(x, np.float32).reshape(T, D)
        top2, p = _route(x2d, np.asarray(gate_w, np.float32),
                         np.asarray(gate_b, np.float32))
        comb = np.zeros((T, N_EXPERTS), dtype=np.float32)
        np.put_along_axis(comb, top2, p, axis=-1)
        out2d += comb @ b2
    return out2d.reshape(S, B, D)


# revision 32
# speedup vs baseline: 1.0179x; 1.0050x over previous
"""MoE layer (8 experts, top-2) on 8 TRN2 NeuronCores — expert parallelism.

Contract: kernel(**inputs) takes FULL inputs, returns FULL output.
Strategy:
  - Host computes the (tiny) gate: logits -> top-2 -> softmax, gathers tokens
    per expert (dispatch), and scatter-adds the scaled expert outputs back
    (combine).  Gate probs are applied on the host during the combine, so the
    device kernel is a plain per-expert FFN.
  - Capacity balancing: per-core columns C are split [0, a) + [a, C).  The A
    range runs the core's own expert; the B range runs spill-over tokens of an
    overloaded expert (second weight set), so C ~ max(1024, fragmentation)
    instead of the max expert load.  (C, a) come from a tiny search over the
    actual expert loads; single-set fallback when no packing helps.
  - Core kernel (bf16 operands, f32 psum): both stages stream the token dim
    as the matmul moving dimension, so PE cost is 512*C cycles exactly:
      phase 1: hT[f, t] = relu(w1.T @ xgt + b1)   (256*C PE cycles)
      phase 2: yT[d, t] = w2-tiles.T @ hT         (256*C PE cycles)
    hT ([128, 32, C] bf16) stays SBUF-resident; w1 streams per F-block in
    phase 1; w2 streams per D-tile in phase 2 from a host-preblocked layout
    ([dt, fi, fo*di]) so every DMA line is 8 KiB.
  - A PE "warmup" chain of dummy matmuls (on a memset tile, no DMA deps)
    runs the p-state clock ramp during the head DMA.

Shapes (hardcoded from the problem spec):
  x [2048, 2, 1024], gate_w [1024, 8], gate_b [8],
  w1 [8, 1024, 4096], b1 [8, 4096], w2 [8, 4096, 1024], b2 [8, 1024].
"""
import sys
import numpy as np

for _p in ("/opt/trn_rl_repo", "/root/.axon_site/_ro/trn_rl_repo"):
    if _p not in sys.path:
        sys.path.insert(0, _p)

import ml_dtypes
import concourse.bacc as bacc
import concourse.tile as tile
import concourse.mybir as mybir
from concourse import bass2jax, mybir as _mybir

N_EXPERTS = 8
TOP_K = 2
S, B, D, F = 2048, 2, 1024, 4096
P = 128
FB = 512                # F-block size streamed through SBUF in phase 1
NB = F // FB            # 8 F-blocks
FC = FB // P            # 4 F-partition-tiles per block
FO = F // P             # 32 F-partition-tiles total
DK = D // P             # 8 contraction tiles for phase 1
DT = D // P             # 8 output D-tiles for phase 2

_f32 = mybir.dt.float32
_bf16 = mybir.dt.bfloat16
_bf16_np = ml_dtypes.bfloat16

_NC_CACHE: dict = {}
_C_MAX = 1664           # max capacity per pass (SBUF budget bound)
LAST_DEVICE_NS = -1     # wall-clock of the last device dispatch (incl. transfers)
LAST_C = -1
LAST_A = -1
LAST_SLOTS = ()


def _c_chunks(C):
    """Split C into chunks <=512 (PSUM bank width in f32), remainder last."""
    out, pos = [], 0
    while C - pos >= 512:
        out.append((pos, 512))
        pos += 512
    if C - pos:
        out.append((pos, C - pos))
    return out


def _slot_assign(surpluses, sizes, n_cores):
    """DFS: per-expert slot counts (k_j slots of sizes[j]) covering each
    surplus, <= n_cores slots of each size total.  Returns per-expert count
    tuples or None."""
    import math
    order = [i for i in range(len(surpluses)) if surpluses[i] > 0]
    ns = len(sizes)

    def dfs(pos, used):
        if pos == len(order):
            return []
        s = surpluses[order[pos]]
        best = None
        # enumerate count tuples for this expert (small search space)
        def opts(rem, j):
            if j == ns - 1:
                k = 0 if rem <= 0 else math.ceil(rem / sizes[j])
                yield (k,)
                return
            kmax = n_cores - used[j]
            for k in range(0, kmax + 1):
                for rest in opts(rem - k * sizes[j], j + 1):
                    yield (k,) + rest
                if rem - k * sizes[j] <= 0:
                    break
        for ks in opts(s, 0):
            if all(used[j] + ks[j] <= n_cores for j in range(ns)):
                sub = dfs(pos + 1, tuple(used[j] + ks[j] for j in range(ns)))
                if sub is not None:
                    return [(order[pos], ks)] + sub
        return best

    return dfs(0, (0,) * ns)


def _pack(loads):
    """Pick (C, a, slots): per-core columns C = a + sum(slots); [0, a) runs
    the core's own expert, each spill slot one (possibly other) expert's
    overflow.  Searches 1- then 2-slot layouts for minimal C."""
    mx = max(loads)
    n = len(loads)
    lo = -(-sum(loads) // n)
    # 1-slot exhaustive
    best1 = None
    for C in range(-(-lo // 4) * 4, mx + 1, 4):
        for a in range(max(C - 512, 1), C):
            bsz = C - a
            if sum(-(-max(0, x - a) // bsz) for x in loads) <= n:
                best1 = (C, a, [bsz])
                break
        if best1:
            break
    # 2-slot search below best1 (spill-w2 prefetch in _build keeps the extra
    # spill streams off the phase-2 tail)
    best = best1
    cap = best1[0] if best1 else mx
    for C in range(-(-lo // 4) * 4, cap, 4):
        found = None
        for a in range(C - 2, max(C - 1024, 0), -1):
            rest = C - a
            for s1 in range(1, rest):
                s2 = rest - s1
                if s1 > s2:
                    continue
                sur = [max(0, x - a) for x in loads]
                if _slot_assign(sur, [s2, s1], n) is not None:
                    found = (C, a, [s2, s1])
                    break
            if found:
                break
        if found:
            best = found
            break
    if best is None or best[0] >= mx:
        C = -(-mx // 4) * 4
        return C, C, []
    return best


def _build(C, a, slots=(), passes=1, *, psum_bufs=8, w1_bufs=2, w2_bufs=2,
           y_bufs=6, n_warm=30, warm_w=128):
    """Trace + compile the per-core SPMD program.

    Columns [0, a) use weight set 0 (the core's own expert); each spill slot
    [a+o, a+o+s) uses its own weight set (inputs w1b/w1c, ...).
    passes>1 repeats the whole compute (same output) — used only for
    differential timing of the device kernel.
    """
    slots = tuple(slots)
    key = (C, a, slots, passes, psum_bufs, w1_bufs, w2_bufs, y_bufs,
           n_warm, warm_w)
    if key in _NC_CACHE:
        return _NC_CACHE[key]
    assert a + sum(slots) == C
    n_sets = 1 + len(slots)
    sfx = ["a", "b", "c"][:n_sets]
    nc = bacc.Bacc("TRN2", target_bir_lowering=False, debug=False,
                   enable_asserts=False, num_devices=8)
    xgt_d = nc.dram_tensor("xgt", (D, C), _bf16, kind="ExternalInput").ap()
    w1_d, w2_d = [], []
    # all sets' b1 packed into one tensor -> one head DMA (HWDGE is 625ns
    # per DMA and the early queue window is congested)
    b1_d = nc.dram_tensor("b1p", (P, n_sets * FO), _f32,
                          kind="ExternalInput").ap()
    for s in sfx:
        w1_d.append(nc.dram_tensor(f"w1{s}", (D, F), _bf16,
                                   kind="ExternalInput").ap())
        # w2 host-preblocked: [dt*fi, fo*di] so per-(dt) DMA lines are 8 KiB
        w2_d.append(nc.dram_tensor(f"w2{s}", (DT * P, FO * P), _bf16,
                                   kind="ExternalInput").ap())
    yT_d = nc.dram_tensor("yT", (D, C), _f32, kind="ExternalOutput").ap()

    xgt_r = xgt_d.rearrange("(ko ki) c -> ki ko c", ki=P)      # [128, 8, C]
    w1_r = [t.rearrange("(ko ki) f -> ki ko f", ki=P) for t in w1_d]
    w2_r = [t.rearrange("(dt fi) fod -> fi dt fod", fi=P) for t in w2_d]
    yT_r = yT_d.rearrange("(do di) c -> di do c", di=P)        # [128, 8, C]

    # chunk = (col offset, size, set idx, offset within the set's hT tile)
    chunks = [(cs, csz, 0, cs) for (cs, csz) in _c_chunks(a)]
    off = a
    for gi, s in enumerate(slots):
        chunks.append((off, s, 1 + gi, 0))
        off += s
    spill_chunks = chunks[len(chunks) - len(slots):]

    with tile.TileContext(nc) as tc:
        with tc.tile_pool(name="const", bufs=1) as cpool, \
             tc.tile_pool(name="w1p", bufs=w1_bufs) as w1pool, \
             tc.tile_pool(name="w2p", bufs=w2_bufs) as w2pool, \
             tc.tile_pool(name="w2s", bufs=4) as w2spool, \
             tc.tile_pool(name="yp", bufs=y_bufs) as ypool, \
             tc.tile_pool(name="ps", bufs=psum_bufs, space="PSUM") as psum:
            xgt_sb = cpool.tile([P, DK, C], _bf16)
            b1_sb = cpool.tile([P, n_sets * FO], _f32)
            # per-set hT tiles: phase-2 groups of one set must not pick up
            # whole-tile dependencies on another set's late relus
            hT_g = [cpool.tile([P, FO, a], _bf16, name="hT_0")]
            for gi, s in enumerate(slots):
                hT_g.append(cpool.tile([P, FO, s], _bf16, name=f"hT_{1+gi}"))

            if n_warm:
                # PE p-state warmup: memset a tile (no DMA dependency) and
                # chain dummy matmuls so the clock ramp runs concurrently
                # with the head DMA.  Result is never read.
                warm = cpool.tile([P, warm_w], _bf16)
                nc.vector.memset(warm[:], 0.0)
                wps = psum.tile([P, warm_w], _f32, tag="ps", name="warm_ps")
                for i in range(n_warm):
                    nc.tensor.matmul(wps[:], warm[:], warm[:],
                                     start=(i == 0), stop=(i == n_warm - 1))

            def relu(fb, fc, ch, ps):
                cs, csz, g, hcs = ch
                fcol = fb * FC + fc
                if g:
                    # spill relus ride the idle DVE: relu(ps + b1) in one
                    # tensor_scalar, so the trailing relus of the last spill
                    # sections don't serialize on ACT ahead of phase 2's
                    # first PSUM-slot reuse
                    nc.vector.tensor_scalar(
                        hT_g[g][:, fcol, hcs:hcs + csz], ps[:, :csz],
                        b1_sb[:, g * FO + fcol:g * FO + fcol + 1], 0.0,
                        mybir.AluOpType.add, mybir.AluOpType.max,
                    )
                else:
                    nc.scalar.activation(
                        hT_g[g][:, fcol, hcs:hcs + csz], ps[:, :csz],
                        mybir.ActivationFunctionType.Relu,
                        bias=b1_sb[:, g * FO + fcol:g * FO + fcol + 1],
                        scale=1.0,
                    )

            def stage1_groups(fb, w1_t, use_chunks, wave_head=False):
                groups = [(fc, ch) for ch in use_chunks for fc in range(FC)]
                if wave_head:
                    # dk-major waves: up to psum_bufs groups accumulate
                    # concurrently so the PE consumes each xgt[dk] as it lands
                    for ws in range(0, len(groups), psum_bufs):
                        wave = groups[ws:ws + psum_bufs]
                        pss = [psum.tile([P, 512], _f32, name=f"ps1h_{ws}_{i}",
                                         tag="ps")
                               for i in range(len(wave))]
                        for dk in range(DK):
                            for (fc, ch), ps in zip(wave, pss):
                                cs, csz, g, _ = ch
                                nc.tensor.matmul(
                                    ps[:, :csz],
                                    w1_t[g][:, dk, fc * P:(fc + 1) * P],
                                    xgt_sb[:, dk, cs:cs + csz],
                                    start=(dk == 0), stop=(dk == DK - 1),
                                )
                        for (fc, ch), ps in zip(wave, pss):
                            relu(fb, fc, ch, ps)
                else:
                    for (fc, ch) in groups:
                        cs, csz, g, _ = ch
                        ps = psum.tile([P, 512], _f32, tag="ps")
                        for dk in range(DK):
                            nc.tensor.matmul(
                                ps[:, :csz],
                                w1_t[g][:, dk, fc * P:(fc + 1) * P],
                                xgt_sb[:, dk, cs:cs + csz],
                                start=(dk == 0), stop=(dk == DK - 1),
                            )
                        relu(fb, fc, ch, ps)

            chunksA_g = chunks[:len(chunks) - len(slots)]

            # spill-set w2 streams through a FIFO pool at ~2-dt prefetch
            # distance; the first PF_DT dts' tiles load during late phase-1
            # sections (the DMA queue has slack there), keeping the spill
            # streams off the phase-2 tail
            PF_DT = 2 if slots else 0
            w2s_tiles = {}

            def w2s_fetch(g, dt):
                t = w2spool.tile([P, FO * P], _bf16, tag="w2s", name="w2s_t")
                nc.sync.dma_start(t[:], w2_r[g][:, dt, :])
                w2s_tiles[(g, dt)] = t

            def phase1(first_rep):
                """Spill work is deferred one section: w1b/w1c(fb) loads ride
                behind w1a(fb+1), and the spill groups run after A(fb+1) —
                keeps the large spill w1 loads out of the congested head DMA
                window and gives each one a full A-section to land."""
                def emit_spill_dmas(pfb):
                    tiles = {}
                    for g in range(1, n_sets):
                        t = w1pool.tile([P, DK, FB], _bf16, tag=f"w1_{g}",
                                        name="w1s_t")
                        nc.sync.dma_start(
                            t[:], w1_r[g][:, :, pfb * FB:(pfb + 1) * FB])
                        tiles[g] = t
                    return tiles

                for fb in range(NB):
                    w1a_t = w1pool.tile([P, DK, FB], _bf16, tag="w1a",
                                        name="w1a_t")
                    if first_rep and fb == 0:
                        # head: per-dk interleave of w1a-block0 and xgt so the
                        # PE starts on dk 0 while later dk slices stream in
                        for dk in range(DK):
                            nc.sync.dma_start(w1a_t[:, dk], w1_r[0][:, dk, 0:FB])
                            nc.sync.dma_start(xgt_sb[:, dk], xgt_r[:, dk])
                        nc.sync.dma_start(b1_sb[:], b1_d)
                    else:
                        nc.sync.dma_start(
                            w1a_t[:], w1_r[0][:, :, fb * FB:(fb + 1) * FB])
                    if slots and fb >= 1:
                        sp_tiles = emit_spill_dmas(fb - 1)
                    pf = fb - (NB - PF_DT * len(slots))
                    if 0 <= pf < PF_DT * len(slots):
                        w2s_fetch(1 + pf % len(slots), pf // len(slots))
                    stage1_groups(fb, [w1a_t], chunksA_g,
                                  wave_head=(first_rep and fb == 0))
                    if slots and fb >= 1:
                        sp = dict(sp_tiles)
                        sp[0] = None
                        stage1_groups(fb - 1, sp, spill_chunks)
                if slots:
                    sp = dict(emit_spill_dmas(NB - 1))
                    sp[0] = None
                    stage1_groups(NB - 1, sp, spill_chunks)

            def phase2():
                """yT[dt, :] = sum_fo w2[fo, dt].T @ hT[fo, :], streamed out."""
                for dt in range(DT):
                    w2a_t = w2pool.tile([P, FO * P], _bf16, tag="w2_0",
                                        name="w2a_t2")
                    nc.sync.dma_start(w2a_t[:], w2_r[0][:, dt, :])
                    if dt + PF_DT < DT:
                        for g in range(1, n_sets):
                            w2s_fetch(g, dt + PF_DT)
                    w2_t = [w2a_t] + [w2s_tiles.pop((g, dt))
                                      for g in range(1, n_sets)]
                    for (cs, csz, g, hcs) in chunks:
                        ps2 = psum.tile([P, 512], _f32, tag="ps")
                        for fo in range(FO):
                            nc.tensor.matmul(
                                ps2[:, :csz],
                                w2_t[g][:, fo * P:(fo + 1) * P],
                                hT_g[g][:, fo, hcs:hcs + csz],
                                start=(fo == 0), stop=(fo == FO - 1),
                            )
                        yt = ypool.tile([P, 512], _f32)
                        nc.vector.tensor_copy(yt[:, :csz], ps2[:, :csz])
                        nc.sync.dma_start(yT_r[:, dt, cs:cs + csz], yt[:, :csz])

            for rep in range(passes):
                phase1(rep == 0)
                phase2()
    nc.compile()
    _NC_CACHE[key] = nc
    return nc


class _Runner:
    """Persistent jitted SPMD executor for a compiled Bacc program.

    Mirrors bass2jax.run_bass_via_pjrt but keeps the jitted callable so
    repeat calls skip retracing/recompiling.
    """

    def __init__(self, nc, n_cores):
        import jax
        from jax.sharding import Mesh, PartitionSpec
        from jax.experimental.shard_map import shard_map

        bass2jax.install_neuronx_cc_hook()
        self.nc = nc
        self.n_cores = n_cores
        in_names, out_names, out_avals = [], [], []
        for alloc in nc.m.functions[0].allocations:
            if not isinstance(alloc, _mybir.MemoryLocationSet):
                continue
            name = alloc.memorylocations[0].name
            if alloc.kind == "ExternalInput":
                in_names.append(name)
            elif alloc.kind == "ExternalOutput":
                out_names.append(name)
                out_avals.append(jax.core.ShapedArray(
                    tuple(alloc.tensor_shape), _mybir.dt.np(alloc.dtype)))
        partition_name = nc.partition_id_tensor.name if nc.partition_id_tensor else None
        in_names = [n for n in in_names if n != partition_name]
        all_names = in_names + out_names + ([partition_name] if partition_name else [])
        self.in_names, self.out_names, self.out_avals = in_names, out_names, out_avals
        self._all_names, self._partition_name = all_names, partition_name
        n_params = len(in_names)

        def _body(*args):
            operands = list(args)
            if partition_name is not None:
                operands.append(bass2jax.partition_id_tensor())
            outs = bass2jax._bass_exec_p.bind(
                *operands,
                out_avals=tuple(out_avals),
                in_names=tuple(all_names),
                out_names=tuple(out_names),
                lowering_input_output_aliases=(),
                sim_require_finite=False,
                sim_require_nnan=False,
                nc=nc,
            )
            return tuple(outs)

        devices = jax.devices()[:n_cores]
        mesh = Mesh(np.asarray(devices), ("core",))
        n_outs = len(out_names)
        self._fn = jax.jit(
            shard_map(_body, mesh=mesh,
                      in_specs=(PartitionSpec("core"),) * (n_params + n_outs),
                      out_specs=(PartitionSpec("core"),) * n_outs,
                      check_rep=False),
            donate_argnums=tuple(range(n_params, n_params + n_outs)),
            keep_unused=True,
        )
        self._jax = jax

    def concat_inputs(self, in_maps):
        return [np.concatenate([np.asarray(m[name]) for m in in_maps], axis=0)
                for name in self.in_names]

    def zero_outs(self):
        jnp = self._jax.numpy
        return [jnp.zeros((self.n_cores * a.shape[0], *a.shape[1:]), a.dtype)
                for a in self.out_avals]

    def run_raw(self, concat_in, zouts):
        outs = self._fn(*concat_in, *zouts)
        self._jax.block_until_ready(outs)
        return outs

    def run(self, in_maps):
        outs = self.run_raw(self.concat_inputs(in_maps), self.zero_outs())
        return [
            {name: np.asarray(outs[i]).reshape(self.n_cores, *self.out_avals[i].shape)[c]
             for i, name in enumerate(self.out_names)}
            for c in range(self.n_cores)
        ]


_RUNNER_CACHE: dict = {}


def _runner(C, a, slots=(), passes=1):
    key = (C, a, tuple(slots), passes)
    if key not in _RUNNER_CACHE:
        _RUNNER_CACHE[key] = _Runner(_build(C, a, slots, passes), N_EXPERTS)
    return _RUNNER_CACHE[key]


def _route(x2d, gate_w, gate_b):
    """Host gate: returns per-token top-2 expert ids and softmax probs (fp32)."""
    logits = x2d.astype(np.float64) @ gate_w.astype(np.float64) + gate_b.astype(np.float64)
    order = np.argsort(-logits, axis=-1, kind="stable")
    top2 = order[:, :TOP_K]                               # [T, 2]
    l = np.take_along_axis(logits, top2, axis=-1)         # [T, 2]
    m = l.max(axis=-1, keepdims=True)
    e = np.exp(l - m)
    p = (e / e.sum(axis=-1, keepdims=True)).astype(np.float32)
    return top2, p


def _block_w2(w2_e_bf):
    """[F, D] -> [dt*fi, fo*di] so per-dt DMA lines are contiguous 8 KiB."""
    return np.ascontiguousarray(
        w2_e_bf.reshape(FO, P, DT, P).transpose(2, 1, 0, 3).reshape(DT * P, FO * P))


def _b1t(b1_e):
    return np.ascontiguousarray(b1_e.astype(np.float32).reshape(FO, P).T)


def prepare(x, gate_w, gate_b, w1, b1, w2):
    """Routing + packing + per-core input maps.  Returns (in_maps, metas,
    C, a, slots) where metas[c] = [(col_off, ix, probs), ...] scatter jobs."""
    T = S * B
    x2d = np.ascontiguousarray(np.asarray(x, np.float32).reshape(T, D))
    top2, p = _route(x2d, np.asarray(gate_w, np.float32),
                     np.asarray(gate_b, np.float32))
    idx_lists = []
    for e in range(N_EXPERTS):
        sel = np.nonzero(top2 == e)          # (token_idx, slot_idx)
        idx_lists.append((sel[0], p[sel[0], sel[1]]))
    loads = [len(ix) for ix, _ in idx_lists]
    C, a, slots = _pack(loads)

    # spill assignment: per slot kind j, per core at most one piece
    # (expert, lo, hi); greedy from the _slot_assign counts
    n_spill = len(slots)
    spill = [[None] * n_spill for _ in range(N_EXPERTS)]
    if n_spill:
        sur = [max(0, n - a) for n in loads]
        counts = _slot_assign(sur, list(slots), N_EXPERTS)
        assert counts is not None
        free = [list(range(N_EXPERTS)) for _ in range(n_spill)]
        for e, ks in counts:
            lo = a
            for j in range(n_spill):
                for _ in range(ks[j]):
                    hi = min(lo + slots[j], loads[e])
                    if lo >= hi:
                        continue
                    c = e if e in free[j] else free[j][0]
                    free[j].remove(c)
                    spill[c][j] = (e, lo, hi)
                    lo = hi
            assert lo >= loads[e]

    xT_bf = np.ascontiguousarray(x2d.T.astype(_bf16_np))  # [D, T] bf16
    w1_bf = [np.ascontiguousarray(np.asarray(w1[e]).astype(_bf16_np))
             for e in range(N_EXPERTS)]
    w2_blk = [_block_w2(np.asarray(w2[e]).astype(_bf16_np))
              for e in range(N_EXPERTS)]
    b1_t = [_b1t(np.asarray(b1[e])) for e in range(N_EXPERTS)]

    sfx = ["a", "b", "c"]
    in_maps, metas = [], []
    for c in range(N_EXPERTS):
        ix_a, p_a = idx_lists[c]
        ix_a, p_a = ix_a[:a], p_a[:a]
        xgt = np.zeros((D, C), dtype=_bf16_np)
        xgt[:, :len(ix_a)] = xT_bf[:, ix_a]
        m = {"xgt": xgt, "w1a": w1_bf[c], "w2a": w2_blk[c]}
        b1_parts = [b1_t[c]]
        jobs = [(0, ix_a, p_a)] if len(ix_a) else []
        off = a
        for j in range(n_spill):
            e = spill[c][j][0] if spill[c][j] else c
            if spill[c][j]:
                _, lo, hi = spill[c][j]
                ix_s, p_s = idx_lists[e][0][lo:hi], idx_lists[e][1][lo:hi]
                xgt[:, off:off + hi - lo] = xT_bf[:, ix_s]
                jobs.append((off, ix_s, p_s))
            m.update({f"w1{sfx[1+j]}": w1_bf[e], f"w2{sfx[1+j]}": w2_blk[e]})
            b1_parts.append(b1_t[e])
            off += slots[j]
        m["b1p"] = np.ascontiguousarray(np.concatenate(b1_parts, axis=1))
        in_maps.append(m)
        metas.append(jobs)
    return in_maps, metas, C, a, slots


def kernel(x, gate_w, gate_b, w1, b1, w2, b2):
    in_maps, metas, C, a, slots = prepare(x, gate_w, gate_b, w1, b1, w2)
    global LAST_C, LAST_A, LAST_SLOTS
    LAST_C, LAST_A, LAST_SLOTS = C, a, slots
    runner = _runner(C, a, slots)

    import time as _time
    _t0 = _time.time()
    results = runner.run(in_maps)
    global LAST_DEVICE_NS
    LAST_DEVICE_NS = int((_time.time() - _t0) * 1e9)

    T = S * B
    out2d = np.zeros((T, D), dtype=np.float32)
    for c in range(N_EXPERTS):
        yT = results[c]["yT"]
        for (off, ix, pr) in metas[c]:
            # combine: scale by gate prob during the scatter-add
            out2d[ix] += pr[:, None] * yT[:, off:off + len(ix)].T

    b2 = np.asarray(b2, np.float32)
    if np.any(b2):
        x2d = np.asarray(x, np.float32).reshape(T, D)
        top2, p = _route(x2d, np.asarray(gate_w, np.float32),
                         np.asarray(gate_b, np.float32))
        comb = np.zeros((T, N_EXPERTS), dtype=np.float32)
        np.put_along_axis(comb, top2, p, axis=-1)
        out2d += comb @ b2
    return out2d.reshape(S, B, D)
